# revision 1
# baseline (speedup 1.0000x reference)
"""Multi-head attention forward on 8 Trainium2 NeuronCores (Bass/Tile).

Problem: B=2, S=2048, d_model=1024, 16 heads (depth 64), fp32.
  q/k/v = query @ W{q,k,v}; logits = q k^T / 8 + mask * -1e9;
  out = softmax(logits) v @ Wo.

Sharding (Megatron-style, hardcoded): core c handles batch b = c//4 and head
group hg = c%4 (4 heads = 256 of the 1024 head dims). Wq/Wk/Wv are
column-sharded, Wo row-sharded; each core emits a partial [S, 1024] output and
the host sums the 4 partials per batch (the "all-reduce").

Per-core kernel design:
  * All attention math runs transposed: qT/kT are [depth, S] (f32r) so QK^T
    lands as logitsT [k, q] tiles straight off the PE with no transposes,
    and AV^T = V.T(lhsT) @ expT needs none either.
  * ScalarE turns logits psum directly into exp weights (scale folded in);
    VectorE multiplies by (1-mask) in bf16 (2x mode). exp weights and V are
    bf16; QK^T and the output projection stay f32r for accuracy.
  * The softmax denominator comes free from a ones-column appended to V
    (row 64 of the AV psum accumulator); reciprocals are computed
    partition-major via tiny PE transposes and broadcast back with a
    rank-1 f32r matmul.
  * The mask streams in 16 split DMAs so the first tiles land early; the
    output projection for each 1024-wide q-chunk is folded into the main
    loop so it overlaps the next chunk's attention; a short identity-matmul
    spin warms the PE clock (HAM) while the first DMAs land.
"""

import sys

import numpy as np

sys.path.insert(0, "/opt/trn_rl_repo")

B = 2
S = 2048
D = 1024
HEADS = 16
DEPTH = 64
CORES = 8
HG = 4          # head groups (cores per batch)
HPC = 4         # heads per core
DH = HPC * DEPTH  # per-core head width = 256

_CACHE = {}


def _build_program():
    import concourse.bass as bass  # noqa: F401  (registers engines)
    import concourse.mybir as mybir
    import concourse.tile as tile
    from concourse import bacc
    from concourse.bass_interp import get_hw_module
    from concourse.masks import make_identity

    dt = mybir.dt
    f32, f32r, bf16 = dt.float32, dt.float32r, dt.bfloat16
    MULT = mybir.AluOpType.mult
    EXP = mybir.ActivationFunctionType.Exp

    nc = bacc.Bacc(
        "TRN2",
        target_bir_lowering=False,
        debug=False,
        enable_asserts=True,
        num_devices=CORES,
    )

    xT = nc.dram_tensor("xT", [D, S], f32r, kind="ExternalInput").ap()
    imaskT = nc.dram_tensor("imaskT", [S, S], bf16, kind="ExternalInput").ap()
    wq = nc.dram_tensor("wq", [D, DH], f32r, kind="ExternalInput").ap()
    wk = nc.dram_tensor("wk", [D, DH], f32r, kind="ExternalInput").ap()
    wv = nc.dram_tensor("wv", [D, DH], f32r, kind="ExternalInput").ap()
    wo = nc.dram_tensor("wo", [DH, D], f32r, kind="ExternalInput").ap()
    vones = nc.dram_tensor("vones", [128, HPC, 1], bf16, kind="ExternalInput").ap()
    ones_rd = nc.dram_tensor("ones_rd", [1, DEPTH], f32r, kind="ExternalInput").ap()
    out = nc.dram_tensor("out", [S, D], f32, kind="ExternalOutput").ap()

    
    with tile.TileContext(nc) as tc:
        with tc.tile_pool(name="persist", bufs=1) as pp:
            # Persistent SBUF tiles.
            qT = [pp.tile([128, S], f32r, tag=f"qT{g}", name=f"qT{g}") for g in range(2)]
            kT = [pp.tile([128, S], f32r, tag=f"kT{g}", name=f"kT{g}") for g in range(2)]
            vt = [pp.tile([128, HPC, DEPTH + 1], bf16, tag=f"v{i}", name=f"v{i}") for i in range(16)]
            wot = [pp.tile([128, D], f32r, tag=f"wo{g}", name=f"wo{g}") for g in range(2)]
            ident = pp.tile([128, 128], f32, tag="ident", name="ident")
            ones_r = pp.tile([1, DEPTH], f32r, tag="ones_r", name="ones_r")
            one1 = pp.tile([1, 1], f32, tag="one1", name="one1")

            make_identity(nc, ident[:])
            with tc.tile_pool(name="psW", bufs=2, space="PSUM") as psW:
                for w in range(80):
                    psw = psW.tile([128, 128], f32, tag="warm", name="warm")
                    nc.tensor.matmul(psw[:], ident[:], ident[:],
                                     start=True, stop=True)
            nc.sync.dma_start(ones_r[:], ones_rd[:])
            nc.gpsimd.memset(one1[:], 1.0)
            for g in range(2):
                nc.sync.dma_start(wot[g][:], wo[g * 128:(g + 1) * 128, :])

            # ---- Phase 1: projections (xT is query[b].T, fed transposed from host)
            with tc.tile_pool(name="xw", bufs=1) as xw, \
                 tc.tile_pool(name="psA", bufs=4, space="PSUM") as psA:
                xt = [xw.tile([128, S], f32r, tag=f"x{d}", name=f"x{d}") for d in range(8)]
                wts = {}
                for nm, srcd in (("wq", wq), ("wk", wk), ("wv", wv)):
                    wts[nm] = [xw.tile([128, DH], f32r, tag=f"{nm}{d}", name=f"{nm}{d}") for d in range(8)]
                for d in range(8):
                    nc.sync.dma_start(wts["wq"][d][:], wq[d * 128:(d + 1) * 128, :])
                for d in range(8):
                    nc.sync.dma_start(xt[d][:], xT[d * 128:(d + 1) * 128, :])
                for nm, srcd in (("wk", wk), ("wv", wv)):
                    for d in range(8):
                        nc.sync.dma_start(wts[nm][d][:], srcd[d * 128:(d + 1) * 128, :])

                # qT/kT: [dh, s] = Wq^T-slice . xT, accumulated over 8 D-chunks.
                for wt, dst in ((wts["wq"], qT), (wts["wk"], kT)):
                    for g in range(2):
                        for sc in range(4):
                            ps = psA.tile([128, 512], f32, tag="proj", name="proj")
                            for d in range(8):
                                nc.tensor.matmul(
                                    ps[:],
                                    wt[d][:, g * 128:(g + 1) * 128],
                                    xt[d][:, sc * 512:(sc + 1) * 512],
                                    start=(d == 0), stop=(d == 7),
                                )
                            nc.vector.tensor_copy(dst[g][:, sc * 512:(sc + 1) * 512], ps[:])

                # v: natural [s, dh] layout, stored per 128-row tile as
                # [128, head, 65] with a ones column at index 64 (denominator).
                for st in range(16):
                    ps = psA.tile([128, DH], f32, tag="proj", name="proj")
                    for d in range(8):
                        nc.tensor.matmul(
                            ps[:],
                            xt[d][:, st * 128:(st + 1) * 128],
                            wts["wv"][d][:],
                            start=(d == 0), stop=(d == 7),
                        )
                    nc.sync.dma_start(
                        vt[st][:, :, DEPTH:DEPTH + 1],
                        vones[:],
                    )
                    nc.vector.tensor_copy(
                        vt[st][:, :, 0:DEPTH],
                        ps[:].rearrange("p (h e) -> p h e", h=HPC),
                    )

            # ---- Phase 2: attention, fully transposed ----
            # Inner loop touches only PE (logits, mask-add via -1e9-identity
            # accumulate, AV^T) and ScalarE (exp psum->sbuf). VectorE only
            # handles the per-head epilogue (attnT copies, reciprocal, norm).
            attnT = [pp.tile([128, S], f32r, tag=f"attnT{g}", name=f"attnT{g}") for g in range(2)]
            with tc.tile_pool(name="attn", bufs=2) as ab, \
                 tc.tile_pool(name="exs", bufs=3) as exs, \
                 tc.tile_pool(name="psL", bufs=2, space="PSUM") as psL, \
                 tc.tile_pool(name="psO", bufs=1, space="PSUM") as psO, \
                 tc.tile_pool(name="psB", bufs=1, space="PSUM") as psB:
                mt = ab.tile([128, 16, S], bf16, tag="mask", name="mask", bufs=1)
                imaskT_r = imaskT.rearrange("(t p) q -> p t q", p=128)
                for kb in range(16):
                    nc.sync.dma_start(mt[:, kb:kb + 1, :], imaskT_r[:, kb:kb + 1, :])
                for qcp in range(2):
                    qs = slice(qcp * 1024, (qcp + 1) * 1024)
                    dden = ab.tile([1, HPC, 1024], f32, tag="dden", name="dden", bufs=1)
                    for h in range(HPC):
                        g, po = h // 2, (h % 2) * 64
                        pso = psO.tile([65, 1024], f32, tag="av", name="av")
                        for kb in range(16):
                            psl = psL.tile([128, 1024], f32, tag="lg", name="lg")
                            for half in range(2):
                                hs = slice(half * 512, (half + 1) * 512)
                                qh = slice(qcp * 1024 + half * 512,
                                           qcp * 1024 + half * 512 + 512)
                                nc.tensor.matmul(
                                    psl[:, hs],
                                    kT[g][po:po + 64, kb * 128:(kb + 1) * 128],
                                    qT[g][po:po + 64, qh],
                                    start=True, stop=True,
                                )
                            ex = exs.tile([128, 1024], bf16, tag="ex", name="ex", bufs=4)
                            nc.scalar.activation(ex[:], psl[:], EXP, scale=0.125)
                            em = exs.tile([128, 1024], bf16, tag="em", name="em", bufs=8)
                            nc.vector.tensor_tensor(em[:], ex[:], mt[:, kb, qs], MULT)
                            for half in range(2):
                                hs = slice(half * 512, (half + 1) * 512)
                                nc.tensor.matmul(
                                    pso[:, hs], vt[kb][:, h, :], em[:, hs],
                                    start=(kb == 0), stop=(kb == 15),
                                )
                        nc.vector.tensor_copy(attnT[g][po:po + 64, qs], pso[0:64, :])
                        nc.vector.tensor_copy(dden[0:1, h, :], pso[64:65, :])

                    # Reciprocal of the 4x1024 denominators: transpose the
                    # single-partition rows into partition-major [128, 32]
                    # columns with tiny PE transposes, reciprocal once, and
                    # transpose back.
                    dflat = dden.rearrange("p h q -> p (h q)")
                    pst = psB.tile([128, 32], f32, tag="dt", name="dt", bufs=2)
                    for c in range(32):
                        nc.tensor.transpose(
                            pst[:, c:c + 1],
                            dflat[0:1, c * 128:(c + 1) * 128],
                            one1[:],
                        )
                    rT = ab.tile([128, 32], f32, tag="rT", name="rT")
                    nc.vector.reciprocal(rT[:], pst[:])
                    rden = [ab.tile([1, 1024], f32r, tag=f"rden{h}", name=f"rden{h}", bufs=1) for h in range(HPC)]
                    for h in range(HPC):
                        g, po = h // 2, (h % 2) * 64
                        for half in range(2):
                            hs = slice(half * 512, (half + 1) * 512)
                            qh = slice(qcp * 1024 + half * 512,
                                       qcp * 1024 + half * 512 + 512)
                            psb = psB.tile([1, 512], f32, tag="dt", name="rdt", bufs=2)
                            for qb in range(4):
                                c = h * 8 + half * 4 + qb
                                nc.tensor.transpose(
                                    psb[0:1, qb * 128:(qb + 1) * 128],
                                    rT[:, c:c + 1],
                                    ident[:],
                                )
                            nc.vector.tensor_copy(rden[h][0:1, hs], psb[:])
                            psc = psB.tile([64, 512], f32, tag="dt", name="psc", bufs=2)
                            nc.tensor.matmul(
                                psc[:], ones_r[:], rden[h][0:1, hs],
                                start=True, stop=True,
                            )
                            nc.vector.tensor_tensor(
                                attnT[g][po:po + 64, qh],
                                attnT[g][po:po + 64, qh], psc[:], MULT,
                            )

                    # Output projection for this qcp's s-range (overlaps
                    # with the next qcp's attention on the other engines).
                    for st in range(qcp * 8, qcp * 8 + 8):
                        ot = ab.tile([128, D], f32, tag="ot", name="ot", bufs=2)
                        for nch in range(2):
                            psf = psB.tile([128, 512], f32, tag="dt", name="po", bufs=2)
                            for g in range(2):
                                nc.tensor.matmul(
                                    psf[:],
                                    attnT[g][:, st * 128:(st + 1) * 128],
                                    wot[g][:, nch * 512:(nch + 1) * 512],
                                    start=(g == 0), stop=(g == 1),
                                )
                            nc.vector.tensor_copy(ot[:, nch * 512:(nch + 1) * 512], psf[:])
                        nc.sync.dma_start(out[st * 128:(st + 1) * 128, :], ot[:])

    nc.compile()
    nc.m = get_hw_module(nc.m)
    return nc


def _get_program():
    if "nc" not in _CACHE:
        _CACHE["nc"] = _build_program()
    return _CACHE["nc"]


def _make_in_maps(query, attention_mask, Wq, Wk, Wv, Wo):
    import ml_dtypes

    in_maps = []
    imaskT_b = []
    xT_b = []
    for b in range(B):
        imaskT_b.append(
            np.ascontiguousarray(1 - attention_mask[b, 0].T).astype(ml_dtypes.bfloat16)
        )
        xT_b.append(np.ascontiguousarray(query[b].T))
    for c in range(CORES):
        b, hg = c // HG, c % HG
        cs = slice(hg * DH, (hg + 1) * DH)
        in_maps.append({
            "xT": xT_b[b],
            "imaskT": imaskT_b[b],
            "wq": np.ascontiguousarray(Wq[:, cs]),
            "wk": np.ascontiguousarray(Wk[:, cs]),
            "wv": np.ascontiguousarray(Wv[:, cs]),
            "wo": np.ascontiguousarray(Wo[cs, :]),
            "vones": np.ones((128, HPC, 1), dtype=ml_dtypes.bfloat16),
            "ones_rd": np.ones((1, DEPTH), dtype=np.float32),
        })
    return in_maps


def _run(inputs, trace=False):
    from concourse.bass_utils import run_bass_kernel_spmd

    nc = _get_program()
    in_maps = _make_in_maps(**inputs)
    res = run_bass_kernel_spmd(
        nc, in_maps, core_ids=list(range(CORES)), trace=trace,
    )
    outs = [res.results[c]["out"].astype(np.float64) for c in range(CORES)]
    full = np.empty((B, S, D), dtype=np.float32)
    for b in range(B):
        acc = outs[4 * b]
        for hg in range(1, HG):
            acc = acc + outs[4 * b + hg]
        full[b] = acc.astype(np.float32)
    return full, res


def kernel(query, attention_mask, Wq, Wk, Wv, Wo):
    full, _ = _run(dict(
        query=np.asarray(query), attention_mask=np.asarray(attention_mask),
        Wq=np.asarray(Wq), Wk=np.asarray(Wk), Wv=np.asarray(Wv),
        Wo=np.asarray(Wo),
    ))
    return full



# revision 9
# speedup vs baseline: 1.1265x; 1.1265x over previous
"""Multi-head attention forward on 8 Trainium2 NeuronCores (Bass/Tile).

Problem: B=2, S=2048, d_model=1024, 16 heads (depth 64), fp32.
  q/k/v = query @ W{q,k,v}; logits = q k^T / 8 + mask * -1e9;
  out = softmax(logits) v @ Wo.

Sharding (Megatron-style, hardcoded): core c handles batch b = c//4 and head
group hg = c%4 (4 heads = 256 of the 1024 head dims). Wq/Wk/Wv are
column-sharded, Wo row-sharded; each core emits a partial [S, 1024] output and
the host sums the 4 partials per batch (the "all-reduce").

v2 design (ScalarE-exp is the roofline: 16.8M exp elems ~ 1 elem/cyc @1.2GHz
= ~110us min; everything else is arranged to hide under it):
  * Inputs stream in bf16 (x, Wq/Wk/Wv) - halves input DMA; q/k kept f32r
    (fp32 psum -> f32r sbuf) for logits accuracy.
  * Projections run as DMA-paced psum chains (each chain accumulates the 8
    d-chunks as they land), so attention starts as soon as the last chunk
    plus ~5us of PE tail is done.
  * Attention inner loop is software-pipelined per (qcp, head-pair):
    QK^T for the two heads of a group are row-packed (K=64 -> row groups 0/64
    run concurrently on the PE), exp runs as 1024-wide ScalarE activations,
    the (1-mask) multiply is a bf16 2x DVE op, and AV lags one kb iteration
    so the PE never waits on the exp.
  * Softmax denominators are free: vt is [128, h, 128] with columns 64:128
    all-ones, so the AV psum rows 64:128 hold the denominator replicated 64x
    (matmul cost depends only on N, not on M). The epilogue is then pure DVE:
    reciprocal_approx_fast on psum rows 64:128 + one fused multiply writing
    normalized attnT. No PE transposes at all.
  * Output projection is a short tail; results go out as fp16 (host upcasts
    and batch-sums in fp32), halving output DMA.
"""

import sys

import numpy as np

sys.path.insert(0, "/opt/trn_rl_repo")

B = 2
S = 2048
D = 1024
HEADS = 16
DEPTH = 64
CORES = 8
HG = 4          # head groups (cores per batch)
HPC = 4         # heads per core
DH = HPC * DEPTH  # per-core head width = 256

_CACHE = {}
DEBUG = False


def _build_program():
    import concourse.bass as bass  # noqa: F401  (registers engines)
    import concourse.mybir as mybir
    import concourse.tile as tile
    from concourse import bacc
    from concourse.bass_interp import get_hw_module
    from concourse.masks import make_identity

    dt = mybir.dt
    f32, f32r, bf16, fp16 = dt.float32, dt.float32r, dt.bfloat16, dt.float16
    MULT = mybir.AluOpType.mult
    EXP = mybir.ActivationFunctionType.Exp

    nc = bacc.Bacc(
        "TRN2",
        target_bir_lowering=False,
        debug=False,
        enable_asserts=True,
        num_devices=CORES,
    )

    xT = nc.dram_tensor("xT", [D, S], bf16, kind="ExternalInput").ap()
    imaskT = nc.dram_tensor("imaskT", [S, S], bf16, kind="ExternalInput").ap()
    wq = nc.dram_tensor("wq", [D, DH], bf16, kind="ExternalInput").ap()
    wk = nc.dram_tensor("wk", [D, DH], bf16, kind="ExternalInput").ap()
    wv = nc.dram_tensor("wv", [D, DH], bf16, kind="ExternalInput").ap()
    wo = nc.dram_tensor("wo", [DH, D], f32r, kind="ExternalInput").ap()
    out = nc.dram_tensor("out", [S, D], fp16, kind="ExternalOutput").ap()
    if DEBUG:
        dbg_qT = nc.dram_tensor("dbg_qT", [2, 128, S], f32r, kind="ExternalOutput").ap()
        dbg_kT = nc.dram_tensor("dbg_kT", [2, 128, S], f32r, kind="ExternalOutput").ap()
        dbg_vt0 = nc.dram_tensor("dbg_vt0", [128, HPC, 128], bf16, kind="ExternalOutput").ap()
        dbg_attnT = nc.dram_tensor("dbg_attnT", [2, 128, S], f32r, kind="ExternalOutput").ap()
        dbg_rden = nc.dram_tensor("dbg_rden", [64, 1024], f32, kind="ExternalOutput").ap()
        dbg_num = nc.dram_tensor("dbg_num", [64, 1024], f32, kind="ExternalOutput").ap()
        dbg_den = nc.dram_tensor("dbg_den", [64, 1024], f32, kind="ExternalOutput").ap()

    with tile.TileContext(nc) as tc:
        with tc.tile_pool(name="persist", bufs=1) as pp:
            # Persistent SBUF tiles.
            qT = [pp.tile([128, S], f32r, tag=f"qT{g}", name=f"qT{g}") for g in range(2)]
            kT = [pp.tile([128, S], f32r, tag=f"kT{g}", name=f"kT{g}") for g in range(2)]
            # vt: per 128-row k-block, per head: cols 0:64 = V, cols 64:128 = 1.0
            # (the ones columns make AV psum rows 64:128 the softmax denominator,
            # replicated across 64 partitions - M doesn't affect matmul time).
            vt = [pp.tile([128, HPC, 128], bf16, tag=f"v{i}", name=f"v{i}") for i in range(16)]
            wot = [pp.tile([128, D], f32r, tag=f"wo{g}", name=f"wo{g}") for g in range(2)]
            attnT = [pp.tile([128, S], f32r, tag=f"attnT{g}", name=f"attnT{g}") for g in range(2)]
            mt = pp.tile([128, 16, S], bf16, tag="mask", name="mask")
            ident = pp.tile([128, 128], f32, tag="ident", name="ident")

            make_identity(nc, ident[:])

            # ones columns of vt (GpSimd, runs during the DMA wait)
            for st in range(16):
                nc.gpsimd.memset(vt[st][:, :, DEPTH:128], 1.0)

            # HAM warmup: ~7us of identity matmuls so the PE is at full clock
            # when the first projection chain fires.
            with tc.tile_pool(name="psW", bufs=2, space="PSUM") as psW:
                for w in range(16):
                    psw = psW.tile([128, 128], f32, tag="warm", name="warm")
                    nc.tensor.matmul(psw[:], ident[:], ident[:],
                                     start=True, stop=True)

            # ---- Phase 1: projections (DMA-paced accumulation chains) ----
            with tc.tile_pool(name="xw", bufs=1) as xw, \
                 tc.tile_pool(name="psA", bufs=4, space="PSUM") as psA, \
                 tc.tile_pool(name="psV", bufs=4, space="PSUM") as psV:
                xt = [xw.tile([128, S], bf16, tag=f"x{d}", name=f"x{d}") for d in range(8)]
                wts = {}
                for nm in ("wq", "wk", "wv"):
                    wts[nm] = [xw.tile([128, DH], bf16, tag=f"{nm}{d}", name=f"{nm}{d}") for d in range(8)]
                # d-interleaved input DMAs: each d-group unlocks the d-th MM of
                # every projection chain.
                for d in range(8):
                    nc.sync.dma_start(wts["wq"][d][:], wq[d * 128:(d + 1) * 128, :])
                    nc.sync.dma_start(wts["wk"][d][:], wk[d * 128:(d + 1) * 128, :])
                    nc.sync.dma_start(xt[d][:], xT[d * 128:(d + 1) * 128, :])
                for d in range(8):
                    nc.sync.dma_start(wts["wv"][d][:], wv[d * 128:(d + 1) * 128, :])
                # mask chunks 0..3 early (needed at attention start), wo, rest.
                imaskT_r = imaskT.rearrange("(t p) q -> p t q", p=128)
                for kb in range(4):
                    nc.sync.dma_start(mt[:, kb:kb + 1, :], imaskT_r[:, kb:kb + 1, :])
                for g in range(2):
                    nc.sync.dma_start(wot[g][:], wo[g * 128:(g + 1) * 128, :])
                for kb in range(4, 16):
                    nc.sync.dma_start(mt[:, kb:kb + 1, :], imaskT_r[:, kb:kb + 1, :])

                # qT/kT: [dh, s] = Wq^T-slice . xT, accumulated over 8 D-chunks.
                # ScalarE drains the psums (it is idle during this phase).
                for sc in range(4):
                    for g in range(2):
                        for wt, dst in ((wts["wq"], qT), (wts["wk"], kT)):
                            ps = psA.tile([128, 512], f32, tag="proj", name="proj")
                            for d in range(8):
                                nc.tensor.matmul(
                                    ps[:],
                                    wt[d][:, g * 128:(g + 1) * 128],
                                    xt[d][:, sc * 512:(sc + 1) * 512],
                                    start=(d == 0), stop=(d == 7),
                                )
                            nc.scalar.copy(dst[g][:, sc * 512:(sc + 1) * 512], ps[:])

                # v: natural [s, dh] layout -> [128, head, 0:64] of vt.
                for st in range(16):
                    ps = psV.tile([128, DH], f32, tag="vproj", name="vproj")
                    for d in range(8):
                        nc.tensor.matmul(
                            ps[:],
                            xt[d][:, st * 128:(st + 1) * 128],
                            wts["wv"][d][:],
                            start=(d == 0), stop=(d == 7),
                        )
                    nc.scalar.copy(
                        vt[st][:, :, 0:DEPTH],
                        ps[:].rearrange("p (h e) -> p h e", h=HPC),
                    )

            # ---- Phase 2: attention, transposed, ScalarE-exp-paced ----
            with tc.tile_pool(name="exs", bufs=4) as exs, \
                 tc.tile_pool(name="eps", bufs=2) as eps, \
                 tc.tile_pool(name="psL", bufs=2, space="PSUM") as psL, \
                 tc.tile_pool(name="psO", bufs=2, space="PSUM") as psO:
                for qcp in range(2):
                    qs = slice(qcp * 1024, (qcp + 1) * 1024)
                    for g in range(2):      # head pair (2g, 2g+1)
                        pso = [psO.tile([128, 1024], f32, tag="av", name=f"av{h}")
                               for h in range(2)]
                        ems = [None, None]
                        for kb in range(16):
                            psl = [None, None]
                            prev_ems = list(ems)
                            for h in range(2):   # h=0 -> rows 0:64, h=1 -> 64:128
                                po = h * 64
                                psl[h] = psL.tile([128, 1024], f32, tag="lg", name="lg")
                                for half in range(2):
                                    hs = slice(half * 512, (half + 1) * 512)
                                    qh = slice(qcp * 1024 + half * 512,
                                               qcp * 1024 + half * 512 + 512)
                                    nc.tensor.matmul(
                                        psl[h][:, hs],
                                        kT[g][po:po + 64, kb * 128:(kb + 1) * 128],
                                        qT[g][po:po + 64, qh],
                                        start=True, stop=True,
                                    )
                            # AV for kb-1 (software pipeline: em is ready).
                            if kb > 0:
                                for h in range(2):
                                    for half in range(2):
                                        hs = slice(half * 512, (half + 1) * 512)
                                        nc.tensor.matmul(
                                            pso[h][:, hs],
                                            vt[kb - 1][:, 2 * g + h, :],
                                            prev_ems[h][:, hs],
                                            start=(kb - 1 == 0), stop=False,
                                        )
                            for h in range(2):
                                ex = exs.tile([128, 1024], bf16, tag="ex", name="ex")
                                nc.scalar.activation(ex[:], psl[h][:], EXP, scale=0.125)
                                em = exs.tile([128, 1024], bf16, tag="em", name="em", bufs=6)
                                nc.vector.tensor_tensor(em[:], ex[:], mt[:, kb, qs], MULT)
                                ems[h] = em
                        # flush AV for kb=15
                        for h in range(2):
                            for half in range(2):
                                hs = slice(half * 512, (half + 1) * 512)
                                nc.tensor.matmul(
                                    pso[h][:, hs],
                                    vt[15][:, 2 * g + h, :],
                                    ems[h][:, hs],
                                    start=False, stop=True,
                                )
                        # Epilogue (pure DVE): rows 64:128 of pso hold the
                        # denominator replicated; normalize rows 0:64 into attnT.
                        for h in range(2):
                            po = h * 64
                            den_s = eps.tile([64, 1024], f32, tag="dens", name="dens")
                            nc.vector.tensor_copy(den_s[:], pso[h][64:128, :])
                            rden = eps.tile([64, 1024], f32, tag="rden", name="rden")
                            nc.vector.reciprocal_approx_fast(rden[:], den_s[:])
                            if DEBUG and qcp == 0 and g == 0 and h == 0:
                                num_s = eps.tile([64, 1024], f32, tag="nums", name="nums")
                                nc.vector.tensor_copy(num_s[:], pso[h][0:64, :])
                                nc.sync.dma_start(dbg_num[:], num_s[:])
                                nc.sync.dma_start(dbg_den[:], den_s[:])
                                nc.sync.dma_start(dbg_rden[:], rden[:])
                            nc.vector.tensor_tensor(
                                attnT[g][po:po + 64, qs],
                                pso[h][0:64, :], rden[:], MULT,
                            )

            if DEBUG:
                for g in range(2):
                    nc.sync.dma_start(dbg_qT[g], qT[g][:])
                    nc.sync.dma_start(dbg_kT[g], kT[g][:])
                    nc.sync.dma_start(dbg_attnT[g], attnT[g][:])
                nc.sync.dma_start(dbg_vt0[:], vt[0][:])

            # ---- Phase 3: output projection tail (fp16 out) ----
            with tc.tile_pool(name="ot", bufs=3) as ob, \
                 tc.tile_pool(name="psF", bufs=2, space="PSUM") as psF:
                for st in range(16):
                    psf = psF.tile([128, D], f32, tag="po", name="po")
                    for nch in range(2):
                        hs = slice(nch * 512, (nch + 1) * 512)
                        for g in range(2):
                            nc.tensor.matmul(
                                psf[:, hs],
                                attnT[g][:, st * 128:(st + 1) * 128],
                                wot[g][:, hs],
                                start=(g == 0), stop=(g == 1),
                            )
                    ot = ob.tile([128, D], fp16, tag="otile", name="otile")
                    if st % 2 == 0:
                        nc.vector.tensor_copy(ot[:], psf[:])
                    else:
                        nc.scalar.copy(ot[:], psf[:])
                    nc.sync.dma_start(out[st * 128:(st + 1) * 128, :], ot[:])

    nc.compile()
    nc.m = get_hw_module(nc.m)
    return nc


def _get_program():
    if "nc" not in _CACHE:
        _CACHE["nc"] = _build_program()
    return _CACHE["nc"]


def _make_in_maps(query, attention_mask, Wq, Wk, Wv, Wo):
    import ml_dtypes

    bf16 = ml_dtypes.bfloat16
    in_maps = []
    imaskT_b = []
    xT_b = []
    for b in range(B):
        imaskT_b.append(
            np.ascontiguousarray(1 - attention_mask[b, 0].T).astype(bf16)
        )
        xT_b.append(np.ascontiguousarray(query[b].T.astype(bf16)))
    for c in range(CORES):
        b, hg = c // HG, c % HG
        cs = slice(hg * DH, (hg + 1) * DH)
        in_maps.append({
            "xT": xT_b[b],
            "imaskT": imaskT_b[b],
            "wq": np.ascontiguousarray(Wq[:, cs].astype(bf16)),
            "wk": np.ascontiguousarray(Wk[:, cs].astype(bf16)),
            "wv": np.ascontiguousarray(Wv[:, cs].astype(bf16)),
            "wo": np.ascontiguousarray(Wo[cs, :]),
        })
    return in_maps


def _run(inputs, trace=False):
    from concourse.bass_utils import run_bass_kernel_spmd

    nc = _get_program()
    in_maps = _make_in_maps(**inputs)
    res = run_bass_kernel_spmd(
        nc, in_maps, core_ids=list(range(CORES)), trace=trace,
    )
    outs = [res.results[c]["out"].astype(np.float32) for c in range(CORES)]
    full = np.empty((B, S, D), dtype=np.float32)
    for b in range(B):
        acc = outs[4 * b]
        for hg in range(1, HG):
            acc = acc + outs[4 * b + hg]
        full[b] = acc
    return full, res


def kernel(query, attention_mask, Wq, Wk, Wv, Wo):
    full, _ = _run(dict(
        query=np.asarray(query), attention_mask=np.asarray(attention_mask),
        Wq=np.asarray(Wq), Wk=np.asarray(Wk), Wv=np.asarray(Wv),
        Wo=np.asarray(Wo),
    ))
    return full


# revision 16
# speedup vs baseline: 1.1307x; 1.0037x over previous
"""Multi-head attention forward on 8 Trainium2 NeuronCores (Bass/Tile).

Problem: B=2, S=2048, d_model=1024, 16 heads (depth 64), fp32.
  q/k/v = query @ W{q,k,v}; logits = q k^T / 8 + mask * -1e9;
  out = softmax(logits) v @ Wo.

Sharding (Megatron-style, hardcoded): core c handles batch b = c//4 and head
group hg = c%4 (4 heads = 256 of the 1024 head dims). Wq/Wk/Wv are
column-sharded, Wo row-sharded; each core emits a partial [S, 1024] output and
the host sums the 4 partials per batch (the "all-reduce").

v2 design (ScalarE-exp is the roofline: 16.8M exp elems ~ 1 elem/cyc @1.2GHz
= ~110us min; everything else is arranged to hide under it):
  * Inputs stream in bf16 (x, Wq/Wk/Wv) - halves input DMA; q/k kept f32r
    (fp32 psum -> f32r sbuf) for logits accuracy.
  * Projections run as DMA-paced psum chains (each chain accumulates the 8
    d-chunks as they land), so attention starts as soon as the last chunk
    plus ~5us of PE tail is done.
  * Attention inner loop is software-pipelined per (qcp, head-pair):
    QK^T for the two heads of a group are row-packed (K=64 -> row groups 0/64
    run concurrently on the PE), exp runs as 1024-wide ScalarE activations,
    the (1-mask) multiply is a bf16 2x DVE op, and AV lags one kb iteration
    so the PE never waits on the exp.
  * Softmax denominators are free: vt is [128, h, 128] with columns 64:128
    all-ones, so the AV psum rows 64:128 hold the denominator replicated 64x
    (matmul cost depends only on N, not on M). The epilogue is then pure DVE:
    reciprocal_approx_fast on psum rows 64:128 + one fused multiply writing
    normalized attnT. No PE transposes at all.
  * Output projection is a short tail; results go out as fp16 (host upcasts
    and batch-sums in fp32), halving output DMA.
"""

import sys

import numpy as np

sys.path.insert(0, "/opt/trn_rl_repo")

B = 2
S = 2048
D = 1024
HEADS = 16
DEPTH = 64
CORES = 8
HG = 4          # head groups (cores per batch)
HPC = 4         # heads per core
DH = HPC * DEPTH  # per-core head width = 256

_CACHE = {}
DEBUG = False


def _build_program():
    import concourse.bass as bass  # noqa: F401  (registers engines)
    import concourse.mybir as mybir
    import concourse.tile as tile
    from concourse import bacc
    from concourse.bass_interp import get_hw_module
    from concourse.masks import make_identity

    dt = mybir.dt
    f32, f32r, bf16, fp16 = dt.float32, dt.float32r, dt.bfloat16, dt.float16
    MULT = mybir.AluOpType.mult
    EXP = mybir.ActivationFunctionType.Exp

    nc = bacc.Bacc(
        "TRN2",
        target_bir_lowering=False,
        debug=False,
        enable_asserts=True,
        num_devices=CORES,
    )

    xT = nc.dram_tensor("xT", [D, S], bf16, kind="ExternalInput").ap()
    imaskT = nc.dram_tensor("imaskT", [S, S], bf16, kind="ExternalInput").ap()
    wq = nc.dram_tensor("wq", [D, DH], bf16, kind="ExternalInput").ap()
    wk = nc.dram_tensor("wk", [D, DH], bf16, kind="ExternalInput").ap()
    wv = nc.dram_tensor("wv", [D, DH], bf16, kind="ExternalInput").ap()
    wo = nc.dram_tensor("wo", [DH, D], f32r, kind="ExternalInput").ap()
    out = nc.dram_tensor("out", [S, D], fp16, kind="ExternalOutput").ap()
    if DEBUG:
        dbg_qT = nc.dram_tensor("dbg_qT", [2, 128, S], f32r, kind="ExternalOutput").ap()
        dbg_kT = nc.dram_tensor("dbg_kT", [2, 128, S], f32r, kind="ExternalOutput").ap()
        dbg_vt0 = nc.dram_tensor("dbg_vt0", [128, HPC, 128], bf16, kind="ExternalOutput").ap()
        dbg_attnT = nc.dram_tensor("dbg_attnT", [2, 128, S], f32r, kind="ExternalOutput").ap()
        dbg_rden = nc.dram_tensor("dbg_rden", [64, 1024], f32, kind="ExternalOutput").ap()
        dbg_num = nc.dram_tensor("dbg_num", [64, 1024], f32, kind="ExternalOutput").ap()
        dbg_den = nc.dram_tensor("dbg_den", [64, 1024], f32, kind="ExternalOutput").ap()

    with tile.TileContext(nc) as tc:
        with tc.tile_pool(name="persist", bufs=1) as pp:
            # Persistent SBUF tiles.
            qT = [pp.tile([128, S], f32r, tag=f"qT{g}", name=f"qT{g}") for g in range(2)]
            kT = [pp.tile([128, S], f32r, tag=f"kT{g}", name=f"kT{g}") for g in range(2)]
            # vt: per 128-row k-block, per head: cols 0:64 = V, cols 64:128 = 1.0
            # (the ones columns make AV psum rows 64:128 the softmax denominator,
            # replicated across 64 partitions - M doesn't affect matmul time).
            vt = [pp.tile([128, HPC, 128], bf16, tag=f"v{i}", name=f"v{i}") for i in range(16)]
            wot = [pp.tile([128, D], f32r, tag=f"wo{g}", name=f"wo{g}") for g in range(2)]
            attnT = [pp.tile([128, S], f32r, tag=f"attnT{g}", name=f"attnT{g}") for g in range(2)]
            mt = pp.tile([128, 16, S], bf16, tag="mask", name="mask")
            ident = pp.tile([128, 128], f32, tag="ident", name="ident")

            identb = pp.tile([128, 128], bf16, tag="identb", name="identb")
            make_identity(nc, ident[:])
            nc.vector.tensor_copy(identb[:], ident[:])

            # ones columns of vt (GpSimd, runs during the DMA wait)
            for st in range(16):
                nc.gpsimd.memset(vt[st][:, :, DEPTH:128], 1.0)

            # HAM warmup: ~7us of identity matmuls so the PE is at full clock
            # when the first projection chain fires.
            with tc.tile_pool(name="psW", bufs=2, space="PSUM") as psW:
                for w in range(16):
                    psw = psW.tile([128, 128], f32, tag="warm", name="warm")
                    nc.tensor.matmul(psw[:], ident[:], ident[:],
                                     start=True, stop=True)

            # ---- Phase 1: projections (DMA-paced accumulation chains) ----
            with tc.tile_pool(name="xw", bufs=1) as xw, \
                 tc.tile_pool(name="psA", bufs=4, space="PSUM") as psA, \
                 tc.tile_pool(name="psV", bufs=4, space="PSUM") as psV:
                xt = [xw.tile([128, S], bf16, tag=f"x{d}", name=f"x{d}") for d in range(8)]
                wts = {}
                for nm in ("wq", "wk", "wv"):
                    wts[nm] = [xw.tile([128, DH], bf16, tag=f"{nm}{d}", name=f"{nm}{d}") for d in range(8)]
                # d-interleaved input DMAs: each d-group unlocks the d-th MM of
                # every projection chain.
                for d in range(8):
                    nc.sync.dma_start(wts["wq"][d][:], wq[d * 128:(d + 1) * 128, :])
                    nc.sync.dma_start(wts["wk"][d][:], wk[d * 128:(d + 1) * 128, :])
                    nc.sync.dma_start(xt[d][:], xT[d * 128:(d + 1) * 128, :])
                for d in range(8):
                    nc.sync.dma_start(wts["wv"][d][:], wv[d * 128:(d + 1) * 128, :])
                # mask chunks 0..3 early (needed at attention start), wo, rest.
                imaskT_r = imaskT.rearrange("(t p) q -> p t q", p=128)
                for kb in range(4):
                    nc.sync.dma_start(mt[:, kb:kb + 1, :], imaskT_r[:, kb:kb + 1, :])
                for g in range(2):
                    nc.sync.dma_start(wot[g][:], wo[g * 128:(g + 1) * 128, :])
                for kb in range(4, 16):
                    nc.sync.dma_start(mt[:, kb:kb + 1, :], imaskT_r[:, kb:kb + 1, :])

                # qT/kT: [dh, s] = Wq^T-slice . xT, accumulated over 8 D-chunks.
                # v chains interleaved 2-per-(q,k) so the PE never idles at the
                # qk->v transition. ScalarE drains the psums (idle this phase).
                def qk_chain(sc, g, wt, dst):
                    ps = psA.tile([128, 512], f32, tag="proj", name="proj")
                    for d in range(8):
                        nc.tensor.matmul(
                            ps[:],
                            wt[d][:, g * 128:(g + 1) * 128],
                            xt[d][:, sc * 512:(sc + 1) * 512],
                            start=(d == 0), stop=(d == 7),
                        )
                    nc.scalar.copy(dst[g][:, sc * 512:(sc + 1) * 512], ps[:])

                def v_chain(st):
                    ps = psV.tile([128, DH], f32, tag="vproj", name="vproj")
                    for d in range(8):
                        nc.tensor.matmul(
                            ps[:],
                            xt[d][:, st * 128:(st + 1) * 128],
                            wts["wv"][d][:],
                            start=(d == 0), stop=(d == 7),
                        )
                    nc.scalar.copy(
                        vt[st][:, :, 0:DEPTH],
                        ps[:].rearrange("p (h e) -> p h e", h=HPC),
                    )

                vst = 0
                for sc in range(4):
                    for g in range(2):
                        qk_chain(sc, g, wts["wq"], qT)
                        qk_chain(sc, g, wts["wk"], kT)
                        v_chain(vst); vst += 1
                        v_chain(vst); vst += 1

            # ---- Phase 2: attention, transposed, ScalarE-exp-paced ----
            with tc.tile_pool(name="exs", bufs=4) as exs, \
                 tc.tile_pool(name="eps", bufs=2) as eps, \
                 tc.tile_pool(name="psL", bufs=2, space="PSUM") as psL, \
                 tc.tile_pool(name="psO", bufs=2, space="PSUM") as psO:
                for qcp in range(2):
                    qs = slice(qcp * 1024, (qcp + 1) * 1024)
                    for g in range(2):      # head pair (2g, 2g+1)
                        # Dummy weight loads bridge the pipeline-fill bubble so
                        # HAM never sees the PE idle at a pair boundary.
                        for _ in range(4 if (qcp, g) != (0, 0) else 8):
                            nc.tensor.ldweights(identb[:])
                        pso = [psO.tile([128, 1024], f32, tag="av", name=f"av{h}")
                               for h in range(2)]
                        ems = [None, None]
                        for kb in range(16):
                            psl = [None, None]
                            prev_ems = list(ems)
                            for h in range(2):   # h=0 -> rows 0:64, h=1 -> 64:128
                                po = h * 64
                                psl[h] = psL.tile([128, 1024], f32, tag="lg", name="lg")
                                for half in range(2):
                                    hs = slice(half * 512, (half + 1) * 512)
                                    qh = slice(qcp * 1024 + half * 512,
                                               qcp * 1024 + half * 512 + 512)
                                    nc.tensor.matmul(
                                        psl[h][:, hs],
                                        kT[g][po:po + 64, kb * 128:(kb + 1) * 128],
                                        qT[g][po:po + 64, qh],
                                        start=True, stop=True,
                                    )
                            # AV for kb-1 (software pipeline: em is ready).
                            if kb > 0:
                                for h in range(2):
                                    for half in range(2):
                                        hs = slice(half * 512, (half + 1) * 512)
                                        nc.tensor.matmul(
                                            pso[h][:, hs],
                                            vt[kb - 1][:, 2 * g + h, :],
                                            prev_ems[h][:, hs],
                                            start=(kb - 1 == 0), stop=False,
                                        )
                            for h in range(2):
                                ex = exs.tile([128, 1024], bf16, tag="ex", name="ex")
                                nc.scalar.activation(ex[:], psl[h][:], EXP, scale=0.125)
                                em = exs.tile([128, 1024], bf16, tag="em", name="em", bufs=6)
                                nc.vector.tensor_tensor(em[:], ex[:], mt[:, kb, qs], MULT)
                                ems[h] = em
                        # flush AV for kb=15
                        for h in range(2):
                            for half in range(2):
                                hs = slice(half * 512, (half + 1) * 512)
                                nc.tensor.matmul(
                                    pso[h][:, hs],
                                    vt[15][:, 2 * g + h, :],
                                    ems[h][:, hs],
                                    start=False, stop=True,
                                )
                        # Epilogue (pure DVE): rows 64:128 of pso hold the
                        # denominator replicated; normalize rows 0:64 into attnT.
                        for h in range(2):
                            po = h * 64
                            den_s = eps.tile([64, 1024], f32, tag="dens", name="dens")
                            nc.vector.tensor_copy(den_s[:], pso[h][64:128, :])
                            rden = eps.tile([64, 1024], f32, tag="rden", name="rden")
                            nc.vector.reciprocal_approx_fast(rden[:], den_s[:])
                            if DEBUG and qcp == 0 and g == 0 and h == 0:
                                num_s = eps.tile([64, 1024], f32, tag="nums", name="nums")
                                nc.vector.tensor_copy(num_s[:], pso[h][0:64, :])
                                nc.sync.dma_start(dbg_num[:], num_s[:])
                                nc.sync.dma_start(dbg_den[:], den_s[:])
                                nc.sync.dma_start(dbg_rden[:], rden[:])
                            nc.vector.tensor_tensor(
                                attnT[g][po:po + 64, qs],
                                pso[h][0:64, :], rden[:], MULT,
                            )

            if DEBUG:
                for g in range(2):
                    nc.sync.dma_start(dbg_qT[g], qT[g][:])
                    nc.sync.dma_start(dbg_kT[g], kT[g][:])
                    nc.sync.dma_start(dbg_attnT[g], attnT[g][:])
                nc.sync.dma_start(dbg_vt0[:], vt[0][:])

            # ---- Phase 3: output projection tail (fp16 out) ----
            with tc.tile_pool(name="ot", bufs=3) as ob, \
                 tc.tile_pool(name="psF", bufs=2, space="PSUM") as psF:
                for st in range(16):
                    psf = psF.tile([128, D], f32, tag="po", name="po")
                    for nch in range(2):
                        hs = slice(nch * 512, (nch + 1) * 512)
                        for g in range(2):
                            nc.tensor.matmul(
                                psf[:, hs],
                                attnT[g][:, st * 128:(st + 1) * 128],
                                wot[g][:, hs],
                                start=(g == 0), stop=(g == 1),
                            )
                    ot = ob.tile([128, D], fp16, tag="otile", name="otile")
                    if st % 2 == 0:
                        nc.vector.tensor_copy(ot[:], psf[:])
                    else:
                        nc.scalar.copy(ot[:], psf[:])
                    nc.sync.dma_start(out[st * 128:(st + 1) * 128, :], ot[:])

    nc.compile()
    nc.m = get_hw_module(nc.m)
    return nc


def _get_program():
    if "nc" not in _CACHE:
        _CACHE["nc"] = _build_program()
    return _CACHE["nc"]


def _make_in_maps(query, attention_mask, Wq, Wk, Wv, Wo):
    import ml_dtypes

    bf16 = ml_dtypes.bfloat16
    in_maps = []
    imaskT_b = []
    xT_b = []
    for b in range(B):
        imaskT_b.append(
            np.ascontiguousarray(1 - attention_mask[b, 0].T).astype(bf16)
        )
        xT_b.append(np.ascontiguousarray(query[b].T.astype(bf16)))
    for c in range(CORES):
        b, hg = c // HG, c % HG
        cs = slice(hg * DH, (hg + 1) * DH)
        in_maps.append({
            "xT": xT_b[b],
            "imaskT": imaskT_b[b],
            "wq": np.ascontiguousarray(Wq[:, cs].astype(bf16)),
            "wk": np.ascontiguousarray(Wk[:, cs].astype(bf16)),
            "wv": np.ascontiguousarray(Wv[:, cs].astype(bf16)),
            "wo": np.ascontiguousarray(Wo[cs, :]),
        })
    return in_maps


def _run(inputs, trace=False):
    from concourse.bass_utils import run_bass_kernel_spmd

    nc = _get_program()
    in_maps = _make_in_maps(**inputs)
    res = run_bass_kernel_spmd(
        nc, in_maps, core_ids=list(range(CORES)), trace=trace,
    )
    outs = [res.results[c]["out"].astype(np.float32) for c in range(CORES)]
    full = np.empty((B, S, D), dtype=np.float32)
    for b in range(B):
        acc = outs[4 * b]
        for hg in range(1, HG):
            acc = acc + outs[4 * b + hg]
        full[b] = acc
    return full, res


def kernel(query, attention_mask, Wq, Wk, Wv, Wo):
    full, _ = _run(dict(
        query=np.asarray(query), attention_mask=np.asarray(attention_mask),
        Wq=np.asarray(Wq), Wk=np.asarray(Wk), Wv=np.asarray(Wv),
        Wo=np.asarray(Wo),
    ))
    return full
